# revision 13
# baseline (speedup 1.0000x reference)
"""Causal multi-headed self-attention (B=2, S=2048, D=1024, H=16, RoPE)
on 8 Trainium2 NeuronCores.

Sharding: tensor-parallel over heads. Each of the 8 cores owns 2 heads
(a contiguous 128-row slice of wq/wk/wv and 128-column slice of wo) and
computes, for both batches, its heads' projections + RoPE + full causal
attention + its partial contribution to the output projection
(out = ctx @ wo.T restricted to its 128 input columns). The host sums
the 8 partial [B*S, D] outputs.

Device-side layout notes:
 - x is fed pre-transposed as xT [D, B*S] bf16 so every matmul has its
   contraction dim on partitions; wq/wk/wv arrive pre-transposed per
   128-row contraction tile (wX3) so no on-device transposes are needed.
 - q,k are produced transposed ([head_dim, B*S]); scores are computed
   transposed (scores.T [Sk, Sq]) so the attention*V matmul needs no
   transposes; V is augmented with a ones column so the softmax
   denominator falls out of the same matmul (row 64 of the accumulator).
 - softmax skips the max-subtraction (scores ~ N(0,1) here; exp is safe
   in fp32) and applies the causal mask as a post-exp multiply on the
   diagonal-band tiles only; fully-masked tiles are never computed.
 - softmax normalization + the output projection run per 512-query
   block, interleaved one block behind the attention inner loop, so the
   Tensor queue never stalls on the normalization chain and the final
   drain tail is one block, not a whole batch.
"""

import numpy as np
import ml_dtypes

import bass_rust
from bass_rust import ScopedClock, VectorClock

import concourse.bass as bass
import concourse.mybir as mybir
import concourse.tile as tile
from concourse.bass_utils import run_bass_kernel_spmd

BF = ml_dtypes.bfloat16
F32 = mybir.dt.float32
BF16 = mybir.dt.bfloat16

B, S, D = 2, 2048, 1024
H = 16
DK = 64
ROPE_THETA = 10000.0
NCORES = 8
BS = B * S            # 4096 rows
HD = 2 * DK           # 128: two heads per core
NT = BS // 512        # 8 column tiles of the transposed activations
KT = D // 128         # 8 contraction tiles
RT = BS // 128        # 32 row tiles
SQT = S // 512        # 4 query tiles per sequence
SKT = S // 128        # 16 key tiles per sequence


class _TileContext(tile.TileContext):
    """TileContext whose exit drain splits its semaphore waits across
    single-wait NOPs — the walrus build in this environment rejects >1
    sync-wait on TPB_CTRL instructions."""

    def _drain_and_barrier(self, tick_clock, wait_clock):
        n_procs = bass_rust.N_PROCS
        gc = tick_clock.global_clock
        ticks = [gc[p] for p in range(n_procs)]
        for p in range(n_procs):
            if ticks[p] <= 0:
                continue
            sub = VectorClock([ticks[q] if q == p else 0 for q in range(n_procs)])
            nop = self.nc.sync.nop(nofuse=True, hint="drain_wait_split")
            wait_clock.add_sem_waits(nop.ins, ScopedClock({None: sub}))
        self.nc.sync.drain()
        self.nc.all_engine_barrier()
        assert self.sems is not None
        popped = self.nc._tile_sem_poison_stack.pop()
        assert popped is self._sem_poison
        self.nc.clear_and_free_semaphores(list(self.sems.allocated().values()))
        self.nc.all_engine_barrier()


_WSPLIT_CTR = [0]


def _split_multi_waits(nc: bass.Bass, max_waits: int = 1):
    """The walrus build here rejects instructions with more than one
    embedded sync wait. Move extra waits onto same-engine NOP carriers
    emitted immediately before the instruction (program order on the
    engine preserves the semantics)."""
    for f in nc.m.functions:
        for bb in f.blocks:
            insts = bb.instructions
            if not any(
                i.sync_info is not None and len(i.sync_info.on_wait) > max_waits
                for i in insts
            ):
                continue
            new = []
            for inst in insts:
                si = inst.sync_info
                if si is not None and len(si.on_wait) > max_waits:
                    waits = list(si.on_wait)
                    for w in waits[:-max_waits]:
                        _WSPLIT_CTR[0] += 1
                        nop = mybir.InstNoOp(
                            name=f"WSPLIT-{_WSPLIT_CTR[0]}", ins=[], outs=[]
                        )
                        nop.engine = inst.engine
                        nop.sync_info = mybir.SyncInfo(on_wait=[w], on_update=[])
                        new.append(nop)
                    inst.sync_info = mybir.SyncInfo(
                        on_wait=waits[-max_waits:], on_update=list(si.on_update)
                    )
                new.append(inst)
            bb.instructions = new


def build_nc() -> bass.Bass:
    nc = bass.Bass()
    xT = nc.declare_dram_parameter("xT", [D, BS], BF16, isOutput=False)
    wq3 = nc.declare_dram_parameter("wq3", [128, D], BF16, isOutput=False)
    wk3 = nc.declare_dram_parameter("wk3", [128, D], BF16, isOutput=False)
    wv3 = nc.declare_dram_parameter("wv3", [128, D], BF16, isOutput=False)
    woT = nc.declare_dram_parameter("woT", [HD, D], BF16, isOutput=False)
    cosT = nc.declare_dram_parameter("cosT", [HD, S], BF16, isOutput=False)
    sinT2 = nc.declare_dram_parameter("sinT2", [HD, S], BF16, isOutput=False)
    maskband = nc.declare_dram_parameter("maskband", [128, 128], BF16, isOutput=False)
    pswap = nc.declare_dram_parameter("pswap", [128, 128], BF16, isOutput=False)
    out = nc.declare_dram_parameter("partial", [BS, D], BF16, isOutput=True)

    MULT = mybir.AluOpType.mult
    ADD = mybir.AluOpType.add
    EXP = mybir.ActivationFunctionType.Exp

    with _TileContext(nc) as tc:
        with (
            tc.tile_pool(name="const", bufs=1) as const,
            tc.tile_pool(name="work", bufs=4) as work,
            tc.tile_pool(name="epool", bufs=6) as epool,
            tc.tile_pool(name="outp", bufs=4) as outp,
            tc.tile_pool(name="zpool", bufs=2) as zpool,
            tc.tile_pool(name="mm", bufs=3, space="PSUM") as mm,
            tc.tile_pool(name="accp", bufs=2, space="PSUM") as accp,
        ):
            # ---- constants / persistent SBUF ----
            # weights + rope tables first (small, on idle queues) so the
            # first projection matmuls are not gated on the bulk x load.
            wq_sb = const.tile([128, KT, HD], BF16)
            wk_sb = const.tile([128, KT, HD], BF16)
            wv_sb = const.tile([128, KT, HD], BF16)
            nc.gpsimd.dma_start(out=wq_sb[:, :, :], in_=wq3[:, :])
            nc.gpsimd.dma_start(out=wk_sb[:, :, :], in_=wk3[:, :])
            nc.gpsimd.dma_start(out=wv_sb[:, :, :], in_=wv3[:, :])
            cos_sb = const.tile([128, S], BF16)
            nc.scalar.dma_start(out=cos_sb, in_=cosT[:, :])
            sin_sb = const.tile([128, S], BF16)
            nc.scalar.dma_start(out=sin_sb, in_=sinT2[:, :])
            pswap_sb = const.tile([128, 128], BF16)
            nc.scalar.dma_start(out=pswap_sb, in_=pswap[:, :])
            mask_sb = const.tile([128, 128], BF16)
            nc.scalar.dma_start(out=mask_sb, in_=maskband[:, :])
            wo_sb = const.tile([128, D], BF16)
            nc.scalar.dma_start(out=wo_sb, in_=woT[:, :])

            xT_sb = const.tile([128, KT, BS], BF16)

            def load_x(b):
                co = b * S
                for kt in range(KT):
                    nc.sync.dma_start(
                        out=xT_sb[:, kt, co : co + S],
                        in_=xT[kt * 128 : (kt + 1) * 128, co : co + S],
                    )

            load_x(0)

            qT_sb = const.tile([128, BS], BF16)
            kT_sb = const.tile([128, BS], BF16)
            ctx_sb = const.tile([128, BS], BF16)
            v_sb = const.tile([128, RT, 130], BF16)
            nc.vector.memset(v_sb[:, :, 64:65], 1.0)
            nc.vector.memset(v_sb[:, :, 129:130], 1.0)
            ones1 = const.tile([1, 64], BF16)
            nc.vector.memset(ones1, 1.0)

            def proj_qk(b):
                co = b * S
                for w_sb, dst in ((wq_sb, qT_sb), (wk_sb, kT_sb)):
                    for jj in range(SQT):
                        cs = co + jj * 512
                        ps = mm.tile([128, 1024], F32, tag="mm", name=f"pj{b}{jj}")
                        psl = ps[:, 0:512]
                        for kt in range(KT):
                            nc.tensor.matmul(
                                psl,
                                lhsT=w_sb[:, kt, :],
                                rhs=xT_sb[:, kt, cs : cs + 512],
                                start=(kt == 0),
                                stop=(kt == KT - 1),
                            )
                        qbf = work.tile([128, 512], BF16, tag="qbf")
                        nc.vector.tensor_copy(qbf, psl)
                        sw = ps[:, 512:1024]
                        nc.tensor.matmul(sw, lhsT=pswap_sb, rhs=qbf, start=True, stop=True)
                        tcs = jj * 512
                        t1 = work.tile([128, 512], F32, tag="t1")
                        nc.gpsimd.tensor_tensor(t1, qbf, cos_sb[:, tcs : tcs + 512], op=MULT)
                        t2 = work.tile([128, 512], F32, tag="t2")
                        nc.vector.tensor_tensor(t2, sw, sin_sb[:, tcs : tcs + 512], op=MULT)
                        nc.gpsimd.tensor_tensor(dst[:, cs : cs + 512], t1, t2, op=ADD)

            def proj_v(b):
                for rt in range(b * SKT, (b + 1) * SKT):
                    ps = mm.tile([128, 1024], F32, tag="mm", name=f"pv{rt}")
                    psv = ps[:, 0:128]
                    for kt in range(KT):
                        nc.tensor.matmul(
                            psv,
                            lhsT=xT_sb[:, kt, rt * 128 : (rt + 1) * 128],
                            rhs=wv_sb[:, kt, :],
                            start=(kt == 0),
                            stop=(kt == KT - 1),
                        )
                    nc.vector.tensor_copy(v_sb[:, rt, 0:64], psv[:, 0:64])
                    nc.vector.tensor_copy(v_sb[:, rt, 65:129], psv[:, 64:128])

            def phase_o_pair(b, j, slot):
                """One (row-tile, half) of the output projection for query
                block j of batch b (ctx columns already normalized)."""
                rt = b * SKT + 4 * j + slot // 2
                od = slot % 2
                ps = mm.tile([128, 512], F32, tag="mm", name=f"po{rt}{od}")
                nc.tensor.matmul(
                    ps,
                    lhsT=ctx_sb[:, rt * 128 : (rt + 1) * 128],
                    rhs=wo_sb[:, od * 512 : (od + 1) * 512],
                    start=True,
                    stop=True,
                )
                ot = outp.tile([128, 512], BF16, tag="o")
                nc.vector.tensor_copy(ot, ps)
                nc.sync.dma_start(
                    out=out[rt * 128 : (rt + 1) * 128, od * 512 : (od + 1) * 512],
                    in_=ot,
                )

            def phase_o_chunk(b, j):
                for slot in range(8):
                    phase_o_pair(b, j, slot)

            def attention(b, trailer):
                """trailer(j) is emitted one block behind: while block
                j+1's inner loop runs, block j's output projection fills
                the Tensor queue without stalling on the normalization."""
                co = b * S
                for j in range(SQT):
                    acc = [
                        accp.tile([65, 512], F32, tag="acc", name=f"acc{b}{j}{hh}")
                        for hh in range(2)
                    ]
                    nsk = 4 * (j + 1)
                    pend = []  # (i, c0, e1) with exp done, AV not yet emitted

                    def emit_av(i, c0, e1):
                        for h in range(2):
                            nc.tensor.matmul(
                                acc[h][:, c0:512],
                                lhsT=v_sb[:, b * SKT + i, 65 * h : 65 * h + 65],
                                rhs=e1[:, 512 * h + c0 : 512 * h + 512],
                                start=(i == 0),
                                stop=(i == nsk - 1),
                            )

                    for i in range(nsk):
                        sk0 = co + i * 128
                        sq0 = co + j * 512
                        t = 128 * i - 512 * j
                        c0 = max(t, 0)  # causally dead column prefix of this tile
                        ps = mm.tile([128, 1024], F32, tag="mm", name=f"sc{b}{j}{i}")
                        # head 0 narrowed to live columns; head 1 full so the
                        # fused exp below reads only initialized psum
                        nc.tensor.matmul(
                            ps[:, c0:512],
                            lhsT=kT_sb[0:DK, sk0 : sk0 + 128],
                            rhs=qT_sb[0:DK, sq0 + c0 : sq0 + 512],
                            start=True,
                            stop=True,
                        )
                        nc.tensor.matmul(
                            ps[:, 512:1024],
                            lhsT=kT_sb[DK : 2 * DK, sk0 : sk0 + 128],
                            rhs=qT_sb[DK : 2 * DK, sq0 : sq0 + 512],
                            start=True,
                            stop=True,
                        )
                        e1 = epool.tile([128, 1024], BF16, tag="E")
                        nc.scalar.activation(e1[:, c0:1024], ps[:, c0:1024], EXP, scale=0.125)
                        if t >= 0:
                            nc.gpsimd.tensor_tensor(
                                e1[:, c0 : c0 + 128], e1[:, c0 : c0 + 128], mask_sb, op=MULT
                            )
                            nc.gpsimd.tensor_tensor(
                                e1[:, 512 + c0 : 512 + c0 + 128],
                                e1[:, 512 + c0 : 512 + c0 + 128],
                                mask_sb,
                                op=MULT,
                            )
                        pend.append((i, c0, e1))
                        if len(pend) > 2:
                            emit_av(*pend.pop(0))
                        # previous block's output projection, one tile per
                        # iteration, riding the attention instruction stream
                        if trailer is not None and j > 0 and i < 8:
                            trailer(j - 1, i)
                    while pend:
                        emit_av(*pend.pop(0))
                    # per-block softmax normalization: 1/Z on DVE straight
                    # from the accumulator's denominator row, broadcast over
                    # 64 partitions with a K=1 matmul, then scale ctx.
                    rz = zpool.tile([1, 1024], BF16, tag="rz", name=f"rz{b}{j}")
                    sq0 = co + j * 512
                    with nc.allow_low_precision(reason="1/Z broadcast in bf16, matches baseline rz path"):
                        for h in range(2):
                            nc.vector.reciprocal(
                                rz[0:1, 512 * h : 512 * h + 512], acc[h][64:65, :]
                            )
                    for h in range(2):
                        zb = mm.tile([64, 512], F32, tag="mm", name=f"zb{b}{h}{j}")
                        nc.tensor.matmul(
                            zb, lhsT=ones1, rhs=rz[0:1, 512 * h : 512 * h + 512],
                            start=True, stop=True,
                        )
                        # DVE reads at most one PSUM operand: stage acc in SBUF
                        ctxu = work.tile([64, 512], F32, tag=f"ctxu{h}", name=f"ctxu{b}{h}{j}")
                        nc.vector.tensor_copy(ctxu, acc[h][0:64, :])
                        nc.vector.tensor_tensor(
                            ctx_sb[DK * h : DK * h + DK, sq0 : sq0 + 512],
                            ctxu,
                            zb,
                            op=MULT,
                        )

            # pipeline: batch-1 projections are emitted after batch-0
            # attention; each batch's output projection is interleaved one
            # query-block behind its attention loop.
            proj_qk(0)
            proj_v(0)
            load_x(1)
            attention(0, lambda j, slot: phase_o_pair(0, j, slot))
            phase_o_chunk(0, SQT - 1)
            proj_qk(1)
            proj_v(1)
            attention(1, lambda j, slot: phase_o_pair(1, j, slot))
            phase_o_chunk(1, SQT - 1)
    return nc


def host_prep(x, wq, wk, wv, wo, token_positions):
    """Build the per-core input maps (host-side shard + layout prep)."""
    x = np.asarray(x, dtype=np.float32)
    xT = x.reshape(BS, D).T.astype(BF)  # [D, BS] contiguous

    pos = np.asarray(token_positions).astype(np.float32)  # [S]
    inv = ROPE_THETA ** (-2.0 * np.arange(DK // 2, dtype=np.float32) / DK)
    freqs = np.outer(pos, inv)  # [S, 32]
    cos = np.cos(freqs)
    sin = np.sin(freqs)
    didx = (np.arange(HD) % DK) // 2
    sign = np.where(np.arange(HD) % 2 == 0, -1.0, 1.0).astype(np.float32)
    cosT = np.ascontiguousarray(cos[:, didx].T).astype(np.float32)  # [128, S]
    sinT2 = np.ascontiguousarray(sin[:, didx].T * sign[:, None]).astype(np.float32)

    ii = np.arange(128)[:, None]
    jj = np.arange(128)[None, :]
    maskband = (ii <= jj).astype(np.float32).astype(BF)  # [128, 128]

    pswap = np.zeros((128, 128), np.float32)
    pswap[np.arange(128) ^ 1, np.arange(128)] = 1.0
    pswap = pswap.astype(BF)

    wq = np.asarray(wq, dtype=np.float32)
    wk = np.asarray(wk, dtype=np.float32)
    wv = np.asarray(wv, dtype=np.float32)
    wo = np.asarray(wo, dtype=np.float32)

    def w3(w, r0):
        # device layout [p, kt, hd]: value = w[r0+hd, kt*128+p]
        arr = w[r0 : r0 + HD, :]  # [128, 1024]
        return np.ascontiguousarray(
            arr.reshape(HD, KT, 128).transpose(2, 1, 0).reshape(128, D)
        ).astype(BF)

    in_maps = []
    for c in range(NCORES):
        r0 = c * HD
        in_maps.append(
            {
                "xT": xT,
                "wq3": w3(wq, r0),
                "wk3": w3(wk, r0),
                "wv3": w3(wv, r0),
                "woT": np.ascontiguousarray(wo[:, r0 : r0 + HD].T).astype(BF),
                "cosT": cosT.astype(BF),
                "sinT2": sinT2.astype(BF),
                "maskband": maskband,
                "pswap": pswap,
            }
        )
    return in_maps


_NC_CACHE = {}


def get_nc() -> bass.Bass:
    if "nc" not in _NC_CACHE:
        nc = build_nc()
        _split_multi_waits(nc)
        _NC_CACHE["nc"] = nc
    return _NC_CACHE["nc"]


def kernel(x, wq, wk, wv, wo, token_positions, **run_kwargs):
    in_maps = host_prep(x, wq, wk, wv, wo, token_positions)
    nc = get_nc()
    res = run_bass_kernel_spmd(nc, in_maps, list(range(NCORES)), **run_kwargs)
    total = np.zeros((BS, D), np.float64)
    for r in res.results:
        total += r["partial"].astype(np.float64)
    out = total.astype(np.float32).reshape(B, S, D)
    if run_kwargs:
        kernel.last_results = res
    return out
